# revision 2
# baseline (speedup 1.0000x reference)
"""VQ-codebook linear layer on 8 Trainium2 NeuronCores.

Problem: out = x_fp16 @ W_q.T where W_q = codebook[indices].reshape(4096, 4096)
  x:        (4, 2048, 4096) fp32
  codebook: (256, 8) fp16
  indices:  (2097152,) int64
  out:      (4, 2048, 4096) fp32

Sharding: tensor-parallel along out_features (column parallel).
Each core computes out[:, c*512:(c+1)*512] = x @ W[c*512:(c+1)*512, :].T

Device kernel (per core), weight-stationary schedule:
  - weight shard resident in SBUF as 32 tiles [128 k, 512 o] fp16 (4 MB)
  - x processed in 4 m-blocks of 2048; per block 4 oc-passes, each pass
    holds stationary W[kc, oc*128:+128] while streaming 4 matmuls of
    [128 k, 512 m] from the block-resident x tiles into 4 PSUM banks
  - bass legalization emits one Ldweights per matmul; a post-legalize
    dedup pass removes consecutive Ldweights with identical APs, so each
    (kc, oc) stationary is loaded once per 4 matmuls (128 vs 4*512 cycles
    -> ~6% PE overhead instead of 25%)
  - x tiles stream on both HWDGE rings (sync=even kc, scalar=odd kc),
    prefetched one block ahead via the FIFO tile pool; next block's x
    DMAs are emitted before the last oc-pass's drains so prefetch isn't
    queued behind drain-gated output DMAs
  - PSUM drains on DVE (fp32 -> fp16 cast), output DMA on the scalar ring,
    out stored transposed [OS, M] fp16 and assembled host-side
"""

import numpy as np

import concourse.bacc as bacc
import concourse.mybir as mybir
import concourse.tile as tile
from concourse import bass_utils

B, S, K = 4, 2048, 4096          # batch, seq, in_features
M = B * S                        # 8192 tokens
O = 4096                         # out_features
CORES = 8
OS = O // CORES                  # 512 out_features per core
KC = K // 128                    # 32 k-chunks
MBW = 2048                       # m-block width
NMB = M // MBW                   # 4 m-blocks
OCC = OS // 128                  # 4 stationary chunks per core
BL = MBW // 512                  # 4 matmuls (psum banks) per stationary

_cached = {}


def _dedup_ldweights(nc):
    """Remove consecutive InstLdweights with identical APs on the PE queue.

    tile_legalize emits one Ldweights before every Matmult; matmuls that
    reuse the same stationary tile don't need the reload (the PE array
    keeps its weights until the next Ldweights).
    """
    removed_total = 0
    for func in nc.m.functions:
        for block in func.blocks:
            insts = list(block.instructions)
            keep = []
            last_sig = None
            removed = {}
            surviving = None
            for inst in insts:
                if isinstance(inst, mybir.InstLdweights):
                    sig = (
                        str(inst.ins[0]),
                        str(inst.perf_mode),
                        str(inst.is_transpose),
                        str(inst.tile_position),
                    )
                    if sig == last_sig:
                        removed[inst.name] = surviving
                        continue
                    last_sig = sig
                    surviving = inst.name
                keep.append(inst)
            if removed:
                block.instructions[:] = keep
                removed_total += len(removed)
                for inst2 in nc.all_instructions():
                    for name, survivor in removed.items():
                        if inst2.has_dependency(name):
                            inst2.remap_dependency_names({name: survivor})
                for name in removed:
                    if name in nc.inst_map:
                        del nc.inst_map[name]
    return removed_total


def _build(repeat=1, xt_bufs=38, dedup=True):
    """Build the per-core program. `repeat` emits the whole compute body
    multiple times inside one NEFF (for benchmarking: tunnel-dispatch
    overhead amortizes across repeats; output is written identically each
    repeat so results are unchanged)."""
    nc = bacc.Bacc("TRN2", target_bir_lowering=False, debug=False)

    xT_d = nc.dram_tensor("xT", [K, M], mybir.dt.float16, kind="ExternalInput")
    wT_d = nc.dram_tensor("wT", [K, OS], mybir.dt.float16, kind="ExternalInput")
    out_d = nc.dram_tensor("out", [OS, M], mybir.dt.float16, kind="ExternalOutput")

    with tile.TileContext(nc) as tc:
        with (
            tc.tile_pool(name="wt", bufs=1) as wt_pool,
            tc.tile_pool(name="xt", bufs=xt_bufs) as xt_pool,
            tc.tile_pool(name="ot", bufs=8) as out_pool,
            tc.tile_pool(name="ps", bufs=8, space="PSUM") as psum_pool,
        ):
            # resident weight shard: 32 tiles [128, 512] fp16 (4 MB)
            wt_tiles = []
            for kc in range(KC):
                wt = wt_pool.tile([128, OS], mybir.dt.float16, tag=f"wt{kc}")
                nc.scalar.dma_start(out=wt[:], in_=wT_d[kc * 128:(kc + 1) * 128, :])
                wt_tiles.append(wt)

            def emit_x(mb):
                tiles = []
                for kc in range(KC):
                    t = xt_pool.tile([128, MBW], mybir.dt.float16, tag="xt")
                    eng = nc.sync if kc % 2 == 0 else nc.scalar
                    eng.dma_start(
                        out=t[:],
                        in_=xT_d[kc * 128:(kc + 1) * 128, mb * MBW:(mb + 1) * MBW],
                    )
                    tiles.append(t)
                return tiles

            flat = [(r, g) for r in range(repeat) for g in range(NMB)]
            nxt = emit_x(flat[0][1])
            for i, (rep, mb) in enumerate(flat):
                cur = nxt
                m0 = mb * MBW
                for oc in range(OCC):
                    psums = [
                        psum_pool.tile(
                            [128, 512], mybir.dt.float32, tag="ps",
                            name=f"ps{rep}_{mb}_{oc}_{b}",
                        )
                        for b in range(BL)
                    ]
                    for kc in range(KC):
                        st = wt_tiles[kc][:, oc * 128:(oc + 1) * 128]
                        for b in range(BL):
                            nc.tensor.matmul(
                                psums[b][:],
                                lhsT=st,
                                rhs=cur[kc][:, b * 512:(b + 1) * 512],
                                start=(kc == 0),
                                stop=(kc == KC - 1),
                            )
                    if oc == OCC - 1 and i + 1 < len(flat):
                        # next block's x DMAs go on the ring queues ahead of
                        # this pass's drain-gated output DMAs
                        nxt = emit_x(flat[i + 1][1])
                    for b in range(BL):
                        o_sb = out_pool.tile(
                            [128, 512], mybir.dt.float16, tag="ot",
                            name=f"ot{rep}_{mb}_{oc}_{b}",
                        )
                        nc.vector.tensor_copy(out=o_sb[:], in_=psums[b][:])
                        nc.scalar.dma_start(
                            out=out_d[oc * 128:(oc + 1) * 128,
                                      m0 + b * 512:m0 + (b + 1) * 512],
                            in_=o_sb[:],
                        )

    if dedup:
        _dedup_ldweights(nc)
    nc.compile()
    return nc


def _prep_inputs(x, codebook, indices):
    codebook = np.asarray(codebook).astype(np.float16, copy=False)
    indices = np.asarray(indices)
    x2 = np.asarray(x).reshape(M, K).astype(np.float16)
    xT = np.ascontiguousarray(x2.T)                       # [K, M] fp16
    W = codebook[indices.astype(np.int64)].reshape(O, K)  # fp16 [4096, 4096]
    in_maps = []
    for c in range(CORES):
        wTc = np.ascontiguousarray(W[c * OS:(c + 1) * OS, :].T)  # [K, OS]
        in_maps.append({"xT": xT, "wT": wTc})
    return in_maps


def kernel(x, codebook, indices):
    if 1 not in _cached:
        _cached[1] = _build(repeat=1)
    nc = _cached[1]
    in_maps = _prep_inputs(x, codebook, indices)
    res = bass_utils.run_bass_kernel_spmd(nc, in_maps, core_ids=list(range(CORES)))
    outT = np.concatenate([res.results[c]["out"] for c in range(CORES)], axis=0)
    out = outT.T.reshape(B, S, O)
    return out.astype(np.float32)
